# revision 73
# baseline (speedup 1.0000x reference)
"""Trainium2 Bass kernel for EvaLinearAttention (nn_EvaLinearAttention_40656160424185).

Strategy: data-parallel over batch B=8 across the 8 NeuronCores (one batch
element per core, no collectives).

Per-core math (x: [N, C], N=4097, C=768, H=12, hd=64):
  qkv = x @ qkv_w.T + bias;  rope on q,k (all tokens but CLS)
  kvT_h = sum_n v_h[n]^T k_roped_h[n]            (pass 1, PSUM-accumulated)
  M_h   = kv_h @ proj_w[:, h].T  -> stacked M [C, C]   (tiny mid phase)
  out   = (q_roped / (hd*N)) @ M + proj_b        (pass 2; attn+proj fused)

Implementation notes (v3; ~184us/core TimelineSim, vs 242us baseline):
  - fp8(e4m3) DoubleRow matmuls with host-side hi/lo error compensation for
    the big x@Wkv projection (x = xh+xl exact; W = Wh + Wl, xl*Wl dropped,
    Wl applied on chunks 0-3 only and skipped on the first 2 tiles while the
    Wl DMA is still in flight).
  - q^T is computed channel-major with Wq-hi only (18 DR matmuls); the rope
    pair rotation (a cross-partition swap) is realized by two strided
    SBUF->SBUF DMA pair-swaps of q^T ⊙ X (X = signed/permuted sin table),
    removing the old second weight matrix. The q bias rides the same PSUM
    accumulation as a 1-partition fp8 hi/lo DR matmul against a ones row, so
    rope(q + qb) comes out of the elementwise work directly (no bias table).
  - k rope: k1 = k ⊙ cos, k2 = k ⊙ skw, k_roped = k1 + pair-swap(k2) (the
    swap is a free-dim AP on the Pool add), so the kvT accumulation needs
    only ONE matmul per 128-channel slice instead of two.
  - every kv 512-col group gets its OWN 1-bank PSUM tile from a 4-deep ring
    and is evicted right after its stop: PSUM WAR tracking is tile-granular,
    so per-group tiles keep the recycle dependency per-group (a shared tile
    serialized the whole pipeline into a convoy).
  - back-work is software-pipelined 1-2 tiles behind the front (k-rope at
    t-1, kvT and q-sum at t-2) so no engine queue head-blocks on a
    cross-engine producer in the same iteration.
  - x and the rope tables ship as ONE per-tile DMA (tables bitcast bf16 in
    the fp8 blob); out rows ship in 2-tile pairs; Wq+qbias share one DMA --
    HWDGE's ~630ns fixed overhead per DMA makes count, not bytes, the
    startup/streaming cost. Startup weight DMAs are JIT-ordered.
  - mid phase streams M chunks out through ACT/DVE while pass-2 tiles 0/1
    accumulate piecewise behind them in the freed qq/kvt PSUM banks; pass-2
    slots rotate over qq/kvt/kv-ring so recycling is 4 tiles deep.
  - M and q_roped are fp8 hi/lo for the DR pass-2; the M residual (lo) is
    kept for contraction chunks 0-3 only (measured +2.2e-3 on the max-rel
    metric, 1.72e-2 total vs the 2e-2 gate). Scales: x*16, W*32, tables
    carry 1/512; output written bf16 scaled by 2^18 (undone on host).
"""

import numpy as np
import ml_dtypes

import concourse.bass as bass  # noqa: F401
import concourse.tile as tile
from concourse import bacc, mybir
from concourse.bass_utils import run_bass_kernel_spmd

F32 = mybir.dt.float32
BF16 = mybir.dt.bfloat16
FP8 = mybir.dt.float8e4
DR = mybir.MatmulPerfMode.DoubleRow

NPF8 = ml_dtypes.float8_e4m3
NPBF = np.dtype(ml_dtypes.bfloat16)

B = 8
N = 4097
NPAD = 4224  # 33 * 128
NT = NPAD // 128
C = 768
H = 12
HD = 64
KC = C // 128  # 6 contraction chunks
NG = 3  # 512-col groups over the 1536 k|v output columns
NRES = 2  # number of chunk-pairs with W-residual compensation (chunks 0-3)
SKIP_RES_TILES = 2  # first tiles run without W-residual (Wl DMA in flight)
SW = 32.0  # weight fp8 scale
SX = 16.0  # x fp8 scale
SS = SW * SX  # 512; combined scale carried by qkv psums
OS = 2.0 ** 18  # output scale (exact, undone on host)

_CACHE = {}


def _build_nc():
    nc = bacc.Bacc("TRN2", target_bir_lowering=False, debug=False, num_devices=B)

    # x (1536B fp8) and the per-tile tables (768B = 384 bf16, bitcast) are
    # packed into ONE per-tile DMA; table cols: ck (64) | skw (64) | cq (128)
    # | xq (128)
    xt8 = nc.dram_tensor("xt8", [128, NT, 2304], FP8, kind="ExternalInput")
    wkvh8 = nc.dram_tensor("wkvh8", [128, NG, KC, 512], FP8, kind="ExternalInput")
    wkvl8 = nc.dram_tensor("wkvl8", [128, NG, 2 * NRES, 512], FP8, kind="ExternalInput")
    # Wq (KC*C) with the fp8 hi/lo q-bias rows appended (partition 0 only)
    wqx8 = nc.dram_tensor("wqx8", [128, KC * C + 2 * C], FP8, kind="ExternalInput")
    vbpb = nc.dram_tensor("vbpb", [1, 2 * C], F32, kind="ExternalInput")
    pw_eff = nc.dram_tensor("pw_eff", [128, KC, C], BF16, kind="ExternalInput")
    out = nc.dram_tensor("out", [NPAD, C], BF16, kind="ExternalOutput")

    with tile.TileContext(nc) as tc:
        with (
            tc.tile_pool(name="const", bufs=1) as const_pool,
            tc.tile_pool(name="wpool", bufs=1) as wpool,
            tc.tile_pool(name="qrs", bufs=1) as qrs_pool,
            tc.tile_pool(name="xin", bufs=5) as xin_pool,
            tc.tile_pool(name="work", bufs=5) as work_pool,
            tc.tile_pool(name="outp", bufs=3) as out_pool,
            tc.tile_pool(name="kvps", bufs=4, space="PSUM") as kv_ps_pool,
            tc.tile_pool(name="qqps", bufs=1, space="PSUM") as qq_ps_pool,
            tc.tile_pool(name="kvtps", bufs=1, space="PSUM") as kvt_ps_pool,
        ):
            # ---- weights / constants resident in SBUF ----
            wkvh_sb = wpool.tile([128, NG, KC, 512], FP8)
            wkvl_sb = wpool.tile([128, NG, 2 * NRES, 512], FP8)
            wqx_sb = wpool.tile([128, KC * C + 2 * C], FP8)
            pw_sb = wpool.tile([128, KC, C], BF16)
            m8h_sb = wpool.tile([128, KC, C], FP8)
            m8l_sb = wpool.tile([128, 4, C], FP8)
            ones_sb = const_pool.tile([1, 2, 128], FP8)
            vbpb_sb = const_pool.tile([128, 2 * C], F32)

            wq_sb = wqx_sb[:, 0 : KC * C].rearrange("p (k c) -> p k c", k=KC)
            qb_sb = wqx_sb[:, KC * C :].rearrange("p (two c) -> p two c", two=2)
            vb_full = vbpb_sb[:, 0:C]
            pb_full = vbpb_sb[:, C : 2 * C]

            qrs = qrs_pool.tile([128, NT, 2, KC, 128], FP8)

            prefetched_x = {}

            def load_x(t):
                xt_sb = xin_pool.tile([128, 2304], FP8, tag="xt8")
                nc.sync.dma_start(xt_sb, xt8.ap()[:, t])
                x_sb = xt_sb[:, 0:1536].rearrange(
                    "p (k two c) -> p k two c", k=KC, two=2
                )
                tt = xt_sb[:, 1536:2304].bitcast(BF16)
                # (x, ck, skw, cq, xq)
                return (x_sb, tt[:, 0:64], tt[:, 64:128], tt[:, 128:256], tt[:, 256:384])

            # startup: JIT-ordered loads. HWDGE has ~630ns fixed overhead per
            # DMA, so keep the critical-path DMA COUNT minimal and ordered by
            # first use: x0, wkvh-g0, wq chunks 0-3, g1, wq rest + qb, g2.
            prefetched_x[0] = load_x(0)
            nc.scalar.dma_start(wkvh_sb[:, 0], wkvh8.ap()[:, 0])
            nc.vector.memset(ones_sb, 1.0)
            nc.scalar.dma_start(wqx_sb[:, 0 : 4 * C], wqx8.ap()[:, 0 : 4 * C])
            nc.scalar.dma_start(wkvh_sb[:, 1], wkvh8.ap()[:, 1])
            nc.scalar.dma_start(wqx_sb[:, 4 * C :], wqx8.ap()[:, 4 * C :])
            nc.scalar.dma_start(wkvh_sb[:, 2], wkvh8.ap()[:, 2])
            prefetched_x[1] = load_x(1)
            nc.scalar.dma_start(wkvl_sb, wkvl8.ap())
            nc.sync.dma_start(vbpb_sb, vbpb.ap().broadcast_to([128, 2 * C]))
            prefetched_x[2] = load_x(2)

            # persistent kvT accumulator: slices 0-3 in bank 0 (cols 0:512),
            # slices 4-5 in bank 1 (cols 512:768, rest junk)
            kvt_ps = kvt_ps_pool.tile([128, 1024], F32, tag="kvt", name="kvt")

            state = {}

            def p1_front(t):
                x_sb, ck, skw, cq, xq = prefetched_x.pop(t, None) or load_x(t)
                res = t >= SKIP_RES_TILES

                kv_sb = work_pool.tile([128, 2 * C], BF16, tag="kvsb")
                qq_ps = qq_ps_pool.tile([128, 1024], F32, tag="qq")
                qq_sb = work_pool.tile([128, C], BF16, tag="qqsb")

                def kv_group(g):
                    # each group gets its own 1-bank PSUM tile so the WAR
                    # dependency on the recycled bank is per-group
                    dst = kv_ps_pool.tile([128, 512], F32, tag="kvg")
                    for c in range(KC):
                        nc.tensor.matmul(
                            dst,
                            x_sb[:, c, :, :],
                            wkvh_sb[:, g, c : c + 1, :].broadcast_to([128, 2, 512]),
                            start=(c == 0),
                            stop=(not res and c == KC - 1),
                            perf_mode=DR,
                        )
                    if res:
                        for cp in range(NRES):
                            nc.tensor.matmul(
                                dst,
                                x_sb[:, 2 * cp : 2 * cp + 2, 0, :],
                                wkvl_sb[:, g, 2 * cp : 2 * cp + 2, :],
                                start=False,
                                stop=(cp == NRES - 1),
                                perf_mode=DR,
                            )
                    # evict this group immediately so the bank recycles early
                    nc.scalar.copy(kv_sb[:, g * 512 : (g + 1) * 512], dst)

                kv_group(0)

                # ---- q^T: out[cq, tok] += Wq[c, cq].T @ x^T[c, tok] + qb ----
                # start=True arms pending-zero bank-wide, so only the first
                # matmul touching each PSUM bank (m=0 -> bank0, m=4 -> bank1)
                # carries it (mirrors the baseline's per-bank start pattern)
                for m in range(KC):
                    dst = qq_ps[:, m * 128 : (m + 1) * 128]
                    for j in range(3):
                        nc.tensor.matmul(
                            dst,
                            wq_sb[:, 2 * j : 2 * j + 2, m * 128 : (m + 1) * 128],
                            x_sb[:, 2 * j : 2 * j + 2, 0, :],
                            start=(m in (0, 4) and j == 0),
                            stop=False,
                            perf_mode=DR,
                        )
                    nc.tensor.matmul(
                        dst,
                        qb_sb[0:1, :, m * 128 : (m + 1) * 128],
                        ones_sb,
                        start=False,
                        stop=True,
                        perf_mode=DR,
                    )
                nc.scalar.copy(qq_sb, qq_ps[:, 0:768])

                kv_group(1)
                kv_group(2)
                state[t] = (kv_sb, qq_sb, ck, skw, cq, xq)

            kstate = {}

            def back_k1(t):
                kv_sb, qq_sb, ck, skw, cq, xq = state[t]
                k_sb = kv_sb[:, 0:768]
                v_sb = kv_sb[:, 768:1536]
                nc.gpsimd.tensor_add(v_sb, v_sb, vb_full)
                # k rope: k_roped = k*ck + pairswap(k*skw); tables carry 1/SS
                ckb = ck.unsqueeze(1).broadcast_to([128, H, 64])
                skb = skw.unsqueeze(1).broadcast_to([128, H, 64])
                k1 = work_pool.tile([128, C], BF16, tag="k1")
                nc.vector.tensor_mul(
                    k1.rearrange("p (h d) -> p h d", h=H),
                    k_sb.rearrange("p (h d) -> p h d", h=H),
                    ckb,
                )
                k2 = work_pool.tile([128, C], BF16, tag="k2")
                nc.vector.tensor_mul(
                    k2.rearrange("p (h d) -> p h d", h=H),
                    k_sb.rearrange("p (h d) -> p h d", h=H),
                    skb,
                )
                if t == NT - 1:
                    k2s = work_pool.tile([128, C], BF16, tag="kr")
                    k2p = k2.rearrange("p (h i two) -> p h i two", h=H, two=2)
                    nc.vector.tensor_copy(
                        k2s.rearrange("p (h i two) -> p h i two", h=H, two=2),
                        k2p[:, :, :, ::-1],
                    )
                    kstate[t] = (v_sb, k1, k2s)
                    return
                kr = work_pool.tile([128, C], BF16, tag="kr")
                krp = kr.rearrange("p (h i two) -> p h i two", h=H, two=2)
                k1p = k1.rearrange("p (h i two) -> p h i two", h=H, two=2)
                k2p = k2.rearrange("p (h i two) -> p h i two", h=H, two=2)
                nc.gpsimd.tensor_add(krp, k1p, k2p[:, :, :, ::-1])
                kstate[t] = (v_sb, kr)

            def back_k2(t):
                ks = kstate.pop(t)
                # kvT accumulation (bf16, contraction over the 128 tokens);
                # the last tile contracts k1 and k2 separately (2 matmuls per
                # slice) so PE need not wait for a Pool add at the p1->mid seam
                v_sb, krs = ks[0], ks[1:]
                for p in range(KC):
                    sl = slice(p * 128, (p + 1) * 128)
                    for ki, kr in enumerate(krs):
                        nc.tensor.matmul(
                            kvt_ps[:, sl],
                            v_sb[:, sl],
                            kr[:, sl],
                            start=(t == 0 and ki == 0 and p in (0, 4)),
                            stop=(
                                t == NT - 1 and ki == len(krs) - 1 and p in (3, 5)
                            ),
                        )

            def back_q1(t):
                kv_sb, qq_sb, ck, skw, cq, xq = state[t]
                cqb = cq.unsqueeze(1).broadcast_to([128, KC, 128])
                xqb = xq.unsqueeze(1).broadcast_to([128, KC, 128])
                q1 = work_pool.tile([128, C], BF16, tag="q1")
                nc.vector.tensor_mul(
                    q1.rearrange("p (j n) -> p j n", j=KC),
                    qq_sb.rearrange("p (j n) -> p j n", j=KC),
                    cqb,
                )
                q2p = work_pool.tile([128, C], BF16, tag="q2p")
                nc.vector.tensor_mul(
                    q2p.rearrange("p (j n) -> p j n", j=KC),
                    qq_sb.rearrange("p (j n) -> p j n", j=KC),
                    xqb,
                )
                # cross-partition pair swap via two strided SBUF->SBUF DMAs
                q2 = work_pool.tile([128, C], BF16, tag="q2")
                q2v = q2.rearrange("(i two) c -> i two c", two=2)
                q2pv = q2p.rearrange("(i two) c -> i two c", two=2)
                nc.sync.dma_start(q2v[:, 0], q2pv[:, 1])
                nc.sync.dma_start(q2v[:, 1], q2pv[:, 0])
                state[t] = (q1, q2)

            def back_q2(t):
                q1, q2 = state.pop(t)
                qsum = work_pool.tile([128, C], BF16, tag="qsum")
                nc.vector.tensor_add(qsum, q1, q2)
                qsum_r = qsum.rearrange("p (j n) -> p j n", j=KC)
                nc.scalar.copy(qrs[:, t, 0], qsum_r)
                nc.vector.tensor_sub(qrs[:, t, 1], qsum_r, qrs[:, t, 0])

            for t in range(NT):
                p1_front(t)
                if t == 4:
                    nc.scalar.dma_start(pw_sb, pw_eff.ap())
                if t >= 1:
                    back_k1(t - 1)
                    back_q1(t - 1)
                if t >= 2:
                    back_k2(t - 2)
                if t >= 2:
                    back_q2(t - 2)
            back_k1(NT - 1)
            back_k2(NT - 2)
            back_q1(NT - 1)
            back_q2(NT - 2)
            back_k2(NT - 1)

            # ---- mid: M[d, c] = sum_e kv[h, d, e] * pw_eff[(h,e), c] ----
            # M chunks stream out through rebalanced ACT/DVE/Pool evictions
            # while pass-2 tiles 0 and 1 accumulate piecewise behind them.
            kvt_sb = wpool.tile([128, C], BF16)
            for p in range(3):
                sl = slice(p * 256, (p + 1) * 256)
                nc.scalar.copy(kvt_sb[:, sl], kvt_ps[:, sl])
            back_q2(NT - 1)

            wslots = {}

            def warm_hi(t, j, stop=False):
                ps = wslots[t]
                for gofs, glen in ((0, 512), (512, 256)):
                    nc.tensor.matmul(
                        ps[:, gofs : gofs + glen],
                        qrs[:, t, :, j, :],
                        m8h_sb[:, j : j + 1, gofs : gofs + glen].broadcast_to(
                            [128, 2, glen]
                        ),
                        start=(j == 0),
                        stop=stop,
                        perf_mode=DR,
                    )

            def warm_lo(t, jp):
                ps = wslots[t]
                for gofs, glen in ((0, 512), (512, 256)):
                    nc.tensor.matmul(
                        ps[:, gofs : gofs + glen],
                        qrs[:, t, 0, jp : jp + 2, :],
                        m8l_sb[:, jp : jp + 2, gofs : gofs + glen],
                        start=False,
                        stop=False,
                        perf_mode=DR,
                    )

            def warm_close(t):
                ps = wslots.pop(t)
                if t == 0:
                    wslots["pair"] = out_pool.tile([128, 2 * C], BF16, tag="osb", name="wpair")
                o_sb = wslots["pair"][:, t * C : (t + 1) * C]
                nc.vector.tensor_add(o_sb, ps[:, 0:768], pb_full)
                if t == 1:
                    nc.scalar.dma_start(
                        out.ap()[0:256, :].rearrange("(two p) c -> p two c", two=2),
                        wslots.pop("pair").rearrange("p (two c) -> p two c", two=2),
                    )

            wslots[0] = qq_ps_pool.tile([128, 1024], F32, tag="qq", name="w0")
            for p in range(KC):
                # two 1-bank slots per chunk: A holds gi 0,1; B holds gi 2
                slot_a = kv_ps_pool.tile([128, 512], F32, tag="kvg")
                slot_b = kv_ps_pool.tile([128, 512], F32, tag="kvg")
                for gi in range(3):
                    slot = slot_a if gi < 2 else slot_b
                    ds = slice((gi % 2) * 256, (gi % 2) * 256 + 256)
                    gs = slice(gi * 256, (gi + 1) * 256)
                    st = gi in (0, 2)
                    sp = gi in (1, 2)
                    nc.tensor.matmul(
                        slot[0:64, ds],
                        kvt_sb[0:64, p * 128 : p * 128 + 64],
                        pw_sb[0:64, p, gs],
                        start=st,
                        stop=sp,
                        tile_position=(0, 0),
                    )
                    nc.tensor.matmul(
                        slot[64:128, ds],
                        kvt_sb[64:128, p * 128 + 64 : p * 128 + 128],
                        pw_sb[64:128, p, gs],
                        start=st,
                        stop=sp,
                        tile_position=(64, 64),
                    )
                nc.scalar.copy(m8h_sb[:, p, 0:512], slot_a)
                nc.vector.tensor_copy(m8h_sb[:, p, 512:768], slot_b[:, 0:256])
                if p < 4:  # M-residual kept for chunks 0-3 only
                    nc.vector.tensor_sub(
                        m8l_sb[:, p, 0:512], slot_a, m8h_sb[:, p, 0:512]
                    )
                    nc.vector.tensor_sub(
                        m8l_sb[:, p, 512:768], slot_b[:, 0:256], m8h_sb[:, p, 512:768]
                    )
                if p == 1:
                    wslots[1] = kvt_ps_pool.tile([128, 1024], F32, tag="kvt", name="w1")
                # warm tiles lag production by 1/3 chunks so their matmuls
                # never stall on the eviction pipeline
                if p >= 1:
                    warm_hi(0, p - 1)
                if p == 2:
                    warm_lo(0, 0)
                if p == 4:
                    warm_lo(0, 2)
                if p >= 3:
                    warm_hi(1, p - 3)
            warm_hi(0, 5, stop=True)
            warm_lo(1, 0)
            warm_hi(1, 3)
            warm_lo(1, 2)
            warm_hi(1, 4)
            warm_hi(1, 5, stop=True)
            warm_close(0)
            warm_close(1)

            # ---- pass 2: out[tok, c] = qr^T.T @ M + pb  (bf16) ----
            # slots rotate over three PSUM resources (qq bank pair, kvt bank
            # pair, kvg 1-bank pair) so WAR recycling is 3 tiles deep
            # out rows for tile pairs ship as ONE DMA (HWDGE overhead amortized)
            o_pair_box = {}

            def p2_front(t):
                paired = t < NT - 3
                if not paired:
                    o_sb = out_pool.tile([128, C], BF16, tag="osb1")
                elif (t - 2) % 2 == 0:
                    o_pair_box["cur"] = out_pool.tile([128, 2 * C], BF16, tag="osb", name="opair")
                    o_sb = o_pair_box["cur"][:, 0:C]
                else:
                    o_sb = o_pair_box["cur"][:, C : 2 * C]
                r = (t - 2) % 4
                if r == 0 and t != NT - 1:
                    big = qq_ps_pool.tile([128, 1024], F32, tag="qq", name="p2q")
                    pieces = ((big[:, 0:512], 0, 512), (big[:, 512:768], 512, 256))
                elif r == 1 and t != NT - 1:
                    big = kvt_ps_pool.tile([128, 1024], F32, tag="kvt", name="p2k")
                    pieces = ((big[:, 0:512], 0, 512), (big[:, 512:768], 512, 256))
                else:
                    big = None
                    pa = kv_ps_pool.tile([128, 512], F32, tag="kvg", name="p2a")
                    pb_ = kv_ps_pool.tile([128, 512], F32, tag="kvg", name="p2b")
                    pieces = ((pa, 0, 512), (pb_[:, 0:256], 512, 256))
                for dst, gofs, glen in pieces:
                    gs = slice(gofs, gofs + glen)
                    for j in range(KC):
                        nc.tensor.matmul(
                            dst,
                            qrs[:, t, :, j, :],
                            m8h_sb[:, j : j + 1, gs].broadcast_to([128, 2, glen]),
                            start=(j == 0),
                            stop=False,
                            perf_mode=DR,
                        )
                    for jp in range(0, 4, 2):
                        nc.tensor.matmul(
                            dst,
                            qrs[:, t, 0, jp : jp + 2, :],
                            m8l_sb[:, jp : jp + 2, gs],
                            start=False,
                            stop=(jp == 2),
                            perf_mode=DR,
                        )
                    if t == NT - 1:
                        # last tile: add + ship each piece as soon as its
                        # accumulation closes to shorten the final chain;
                        # pieces go to different queues to overlap issue
                        nc.vector.tensor_add(
                            o_sb[:, gs], dst, pb_full[:, gs]
                        )
                        q = nc.scalar if gofs == 0 else nc.sync
                        q.dma_start(
                            out.ap()[t * 128 : (t + 1) * 128, gs], o_sb[:, gs]
                        )
                if t == NT - 1:
                    return
                if big is not None:
                    nc.vector.tensor_add(o_sb, big[:, 0:768], pb_full)
                else:
                    nc.vector.tensor_add(o_sb[:, 0:512], pa, pb_full[:, 0:512])
                    nc.vector.tensor_add(
                        o_sb[:, 512:768], pb_[:, 0:256], pb_full[:, 512:768]
                    )
                if not paired:
                    nc.scalar.dma_start(out.ap()[t * 128 : (t + 1) * 128, :], o_sb)
                elif (t - 2) % 2 == 1:
                    t0 = t - 1
                    nc.scalar.dma_start(
                        out.ap()[t0 * 128 : (t0 + 2) * 128, :].rearrange(
                            "(two p) c -> p two c", two=2
                        ),
                        o_pair_box["cur"].rearrange("p (two c) -> p two c", two=2),
                    )

            for t in range(2, NT):
                p2_front(t)

    nc.compile()
    return nc


def _prep_inputs(x, rope, qkv_w, q_bias, v_bias, proj_w, proj_b):
    f = np.float32

    sin = rope[:, :HD].astype(f)
    cos = rope[:, HD:].astype(f)
    cfull = np.zeros((NPAD, HD), f)
    cfull[0] = 1.0
    cfull[1:N] = cos
    sfull = np.zeros((NPAD, HD), f)
    sfull[1:N] = sin

    # skw[dd]: dd=2i -> +sin[2i+1], dd=2i+1 -> -sin[2i]   (all / SS)
    skw = np.empty((NPAD, HD), f)
    skw[:, 0::2] = sfull[:, 1::2]
    skw[:, 1::2] = -sfull[:, 0::2]

    # k tables (token-major): ck | skw
    ktab = np.concatenate([cfull / SS, skw / SS], axis=1)  # [NPAD, 128]

    # q tables (channel-major): cq^T | xq^T; partition p reads row p%64
    qtab = np.empty((NT, 128, 256), f)
    for t in range(NT):
        blk_c = cfull[t * 128 : (t + 1) * 128].T / SS  # [64, 128]
        blk_x = skw[t * 128 : (t + 1) * 128].T / SS  # [64, 128]
        qtab[t, :, 0:128] = np.tile(blk_c, (2, 1))
        qtab[t, :, 128:256] = np.tile(blk_x, (2, 1))

    tabs = np.concatenate([ktab.reshape(NT, 128, 128), qtab], axis=2).astype(NPBF)

    wt = np.ascontiguousarray(qkv_w.T.astype(f))  # [C, 3C]
    Wq, Wkv = wt[:, :C], wt[:, C:]

    kvh = (Wkv * SW).astype(NPF8)
    kvl = ((Wkv * SW) - kvh.astype(f)).astype(NPF8)
    hi4 = kvh.reshape(KC, 128, NG, 512).transpose(1, 2, 0, 3)  # [128, NG, KC, 512]
    lo4 = kvl.reshape(KC, 128, NG, 512).transpose(1, 2, 0, 3)
    wkvh8 = np.ascontiguousarray(hi4)
    wkvl8 = np.ascontiguousarray(lo4[:, :, : 2 * NRES, :])

    wq8 = (Wq * SW).astype(NPF8).reshape(KC, 128, C).transpose(1, 0, 2)  # [128,KC,C]

    pw = proj_w.T.astype(f) * (OS / (HD * N) / SS)
    pw_eff = np.ascontiguousarray(
        pw.reshape(KC, 128, C).transpose(1, 0, 2).astype(NPBF)
    )

    qbs = q_bias.astype(f) * SS
    qbh = qbs.astype(NPF8)
    qbl = (qbs - qbh.astype(f)).astype(NPF8)
    qbrow = np.concatenate([qbh, qbl])[None, :]  # [1, 2C]
    wqx8 = np.concatenate(
        [wq8.reshape(128, KC * C), np.broadcast_to(qbrow, (128, 2 * C))], axis=1
    )

    common = dict(
        wkvh8=wkvh8,
        wkvl8=wkvl8,
        wqx8=np.ascontiguousarray(wqx8),
        vbpb=np.ascontiguousarray(
            np.concatenate([v_bias.astype(f) * SS, proj_b.astype(f) * OS])[None, :]
        ),
        pw_eff=pw_eff,
    )

    tabs_bytes = np.ascontiguousarray(tabs.transpose(1, 0, 2)).view(np.uint8)
    tabs_bytes = tabs_bytes.reshape(128, NT, 768)  # [p, t, bytes]

    in_maps = []
    for b in range(B):
        xs = np.zeros((NPAD, C), f)
        xs[:N] = x[b] * SX
        xh = xs.astype(NPF8)
        xl = (xs - xh.astype(f)).astype(NPF8)
        stacked = np.stack([xh, xl], axis=0).reshape(2, NT, 128, KC, 128)
        xbytes = (
            np.ascontiguousarray(stacked.transpose(4, 1, 3, 0, 2))
            .view(np.uint8)
            .reshape(128, NT, 1536)
        )
        m = dict(common)
        m["xt8"] = np.ascontiguousarray(
            np.concatenate([xbytes, tabs_bytes], axis=2)
        ).view(NPF8)
        in_maps.append(m)
    return in_maps


def kernel(x, rope, qkv_w, q_bias, v_bias, proj_w, proj_b, _trace=False):
    x = np.asarray(x, dtype=np.float32)
    rope = np.asarray(rope, dtype=np.float32)
    qkv_w = np.asarray(qkv_w, dtype=np.float32)
    q_bias = np.asarray(q_bias, dtype=np.float32)
    v_bias = np.asarray(v_bias, dtype=np.float32)
    proj_w = np.asarray(proj_w, dtype=np.float32)
    proj_b = np.asarray(proj_b, dtype=np.float32)
    if "nc" not in _CACHE:
        _CACHE["nc"] = _build_nc()
    nc = _CACHE["nc"]
    in_maps = _prep_inputs(x, rope, qkv_w, q_bias, v_bias, proj_w, proj_b)
    res = run_bass_kernel_spmd(nc, in_maps, core_ids=list(range(B)), trace=_trace)
    out = np.stack(
        [res.results[b]["out"][:N].astype(np.float32) for b in range(B)], axis=0
    )
    if _trace:
        _CACHE["last_result"] = res
    return out * np.float32(1.0 / OS)
